# revision 21
# baseline (speedup 1.0000x reference)
"""Trainium2 Bass kernel for nn_AttentionType1 (S=1024, E=1024, H=16, HD=64).

Tensor-parallel over heads, 2 heads per core on 8 NeuronCores.

Per core c (heads 2c, 2c+1):
  - Projections (bf16, weight slices stationary): newQT = (Wq_c @ q.T + q_emb)*scale,
    KT = Wk_c @ k.T (both [128, S], head-dim on partitions), V natural [t, d].
  - Scores [s, t] per (head, s-chunk) in PSUM: s1 via QK matmul (k=64 per head,
    both heads row-packed in the PE) plus the relative/speaker term folded into
    the TensorEngine as two diagonal-stationary matmuls:
    s2 = diag(d0) @ utt + diag(d1-d0) @ (spk*utt).
  - Mask + PSUM eviction fused: one scalar_tensor_tensor multiplies by keep
    (1-mask) while moving PSUM->SBUF fp16 (reference's 1e-30 equals 0.0 under
    exp in fp32).
  - Softmax: fused exp + row-sum on ScalarE (accum_out), no max-subtraction
    (logits bounded ~|8|), normalize P by 1/Z per-partition (tensor_scalar 4x).
  - P transposed via DMA-xbar transpose (bf16) into [t', tc, s] tiles.
  - PV: V tiles stationary, both heads packed into one PSUM via column tiling.
  - Output: AllGather the tiny attn_out.T (bf16) in two s-halves; AG of half 0
    overlaps the scores of s-chunks 4..7. Each core then computes a distinct
    128-row slice of out.T = Wo @ attn_out.T locally -- no all-reduce.
Host does layout-only prep (transpose/reshape/cast) and concatenation.
"""

import sys

if "/opt/trn_rl_repo" not in sys.path:
    sys.path.insert(0, "/opt/trn_rl_repo")

import numpy as np
import ml_dtypes

S = 1024
E = 1024
H = 16
HD = 64
N_CORES = 8
P = 128
SCALE = float(HD) ** -0.5  # 0.125

_CACHE = {}
LAST_EXEC_NS = None
TRACE = False
TRACE_DIR = None


def _build():
    if "nc" in _CACHE:
        return _CACHE["nc"]

    import concourse.mybir as mybir
    import concourse.tile as tile
    from concourse import bacc
    from concourse.masks import make_identity

    f32 = mybir.dt.float32
    bf16 = mybir.dt.bfloat16
    fp16 = mybir.dt.float16
    u8 = mybir.dt.uint8
    AF = mybir.ActivationFunctionType
    ALU = mybir.AluOpType

    nc = bacc.Bacc("TRN2", target_bir_lowering=False, debug=False,
                   num_devices=N_CORES)

    # --- external IO (per-core shards, host-prepped layouts) ---
    qt_e = nc.dram_tensor("qt", [P, 8, S], bf16, kind="ExternalInput").ap()
    kt_e = nc.dram_tensor("kt", [P, 8, S], bf16, kind="ExternalInput").ap()
    vt_e = nc.dram_tensor("vt", [P, 8, S], bf16, kind="ExternalInput").ap()
    wq_e = nc.dram_tensor("wq", [P, 8, P], bf16, kind="ExternalInput").ap()
    wk_e = nc.dram_tensor("wk", [P, 8, P], bf16, kind="ExternalInput").ap()
    wv_e = nc.dram_tensor("wv", [P, 8, P], bf16, kind="ExternalInput").ap()
    wo_e = nc.dram_tensor("wo", [P, 8, P], bf16, kind="ExternalInput").ap()
    utt_e = nc.dram_tensor("utt", [P, 8, S], bf16, kind="ExternalInput").ap()
    spk_e = nc.dram_tensor("spk", [P, 8, S], u8, kind="ExternalInput").ap()
    kp_e = nc.dram_tensor("kp", [P, 16, S], u8, kind="ExternalInput").ap()
    enc_e = nc.dram_tensor("enc", [P, 2], bf16, kind="ExternalInput").ap()
    encq_e = nc.dram_tensor("encq", [P, 1], f32, kind="ExternalInput").ap()
    out_e = nc.dram_tensor("out", [P, S], f32, kind="ExternalOutput").ap()

    class _NoAddSet(set):
        def add(self, x):  # noqa: ARG002
            pass

    with tile.TileContext(nc) as tc:
        # Collectives here only touch DRAM buffers that no DMA-transpose ever
        # reads or writes; skip the global transpose<->collective
        # serialization, which otherwise stalls the softmax pipeline behind
        # every AllGather.
        tc.serialize_transpose_collective_names = _NoAddSet()
        with tc.tile_pool(name="const", bufs=1) as const, \
             tc.tile_pool(name="pers", bufs=1) as pers, \
             tc.tile_pool(name="work", bufs=2) as work, \
             tc.tile_pool(name="ps_big", bufs=2, space="PSUM") as ps_big, \
             tc.tile_pool(name="ps_sm", bufs=2, space="PSUM") as ps_sm, \
             tc.tile_pool(name="ps_o", bufs=1, space="PSUM") as ps_o, \
             tc.tile_pool(name="dram", bufs=1, space="DRAM") as dram:

            ident = const.tile([P, P], bf16)
            make_identity(nc, ident[:])
            enc_sb = const.tile([P, 2], bf16)
            nc.sync.dma_start(enc_sb[:], enc_e[:])
            encq_sb = const.tile([P, 1], f32)
            nc.sync.dma_start(encq_sb[:], encq_e[:])
            ebias = const.tile([P, 1], f32)
            nc.vector.tensor_scalar_mul(ebias[:], encq_sb[:], SCALE)
            enc2 = const.tile([P, 2], bf16)
            nc.vector.tensor_copy(enc2[:, 0:1], enc_sb[:, 0:1])
            nc.vector.tensor_sub(enc2[:, 1:2], enc_sb[:, 1:2], enc_sb[:, 0:1])

            newqt = pers.tile([P, S], bf16)
            ktc = pers.tile([P, S], bf16)
            v_sb = pers.tile([P, 8, P], bf16)      # [t', tc, d(2 heads)]
            utt_sb = pers.tile([P, 8, S], bf16)    # [p, i, t], s = i*128+p
            w_sb = pers.tile([P, 8, S], bf16)      # spk*utt
            kp_sb = pers.tile([P, 16, S], u8)      # keep = 1-mask, [p, 8h+i, t]
            dots_sb = pers.tile([P, 8, 4], f32)    # [p, i, 2h+v]
            wo_sb = pers.tile([P, 8, P], bf16)
            pt0 = pers.tile([P, 8, S], bf16)       # P.T head0: [t', tc, s]
            pt1 = pers.tile([P, 8, S], bf16)
            pts = (pt0, pt1)

            # tiny warmup AllGather (content irrelevant): absorbs the
            # collectives first-call cost under the input-DMA phase
            wu_in = dram.tile([P, 2], bf16, name="wu_in")
            wu_out = dram.tile([N_CORES * P, 2], bf16, addr_space="Shared",
                               name="wu_out")
            nc.gpsimd.collective_compute(
                "AllGather", mybir.AluOpType.bypass,
                replica_groups=[list(range(N_CORES))],
                ins=[wu_in.opt()], outs=[wu_out.opt()])
            # DRAM bounce buffers for the two AllGather halves
            at_d = [dram.tile([P, 512], bf16, name=f"at_d{g}") for g in range(2)]
            ag_d = [dram.tile([N_CORES * P, 512], bf16, addr_space="Shared",
                              name=f"ag_d{g}") for g in range(2)]

            # ---------- input DMAs (ordered by first use) ----------
            with tc.tile_pool(name="setup", bufs=1) as setup:
                wq_sb = setup.tile([P, 8, P], bf16)
                nc.sync.dma_start(wq_sb[:], wq_e[:])
                wk_sb = setup.tile([P, 8, P], bf16)
                nc.sync.dma_start(wk_sb[:], wk_e[:])
                qt_sb = setup.tile([P, 8, S], bf16)
                kt_sb = setup.tile([P, 8, S], bf16)
                for g in range(2):
                    gs = slice(g * 4, (g + 1) * 4)
                    nc.sync.dma_start(qt_sb[:, gs, :], qt_e[:, gs, :])
                    nc.sync.dma_start(kt_sb[:, gs, :], kt_e[:, gs, :])
                spk_sb = setup.tile([P, 8, S], u8)
                vt_sb = setup.tile([P, 8, S], bf16)
                wv_sb = setup.tile([P, 8, P], bf16)
                # scalar ring: mask/speaker inputs per s-chunk, by first use
                for i in range(8):
                    nc.scalar.dma_start(utt_sb[:, i:i + 1, :],
                                        utt_e[:, i:i + 1, :])
                    nc.scalar.dma_start(spk_sb[:, i:i + 1, :],
                                        spk_e[:, i:i + 1, :])
                    nc.scalar.dma_start(kp_sb[:, i:i + 1, :],
                                        kp_e[:, i:i + 1, :])
                    nc.scalar.dma_start(kp_sb[:, 8 + i:9 + i, :],
                                        kp_e[:, 8 + i:9 + i, :])
                    nc.gpsimd.tensor_mul(w_sb[:, i, :], spk_sb[:, i, :],
                                         utt_sb[:, i, :])
                    if i == 3:
                        nc.scalar.dma_start(vt_sb[:], vt_e[:])
                        nc.scalar.dma_start(wv_sb[:], wv_e[:])
                nc.scalar.dma_start(wo_sb[:], wo_e[:])

                # ---------- phase 0: projections ----------
                for n in range(2):
                    sl = slice(n * 512, (n + 1) * 512)
                    pq = ps_sm.tile([P, 512], f32, tag="pp")
                    for kk in range(8):
                        nc.tensor.matmul(pq[:], wq_sb[:, kk, :],
                                         qt_sb[:, kk, sl],
                                         start=(kk == 0), stop=(kk == 7))
                    nc.scalar.activation(newqt[:, sl], pq[:], AF.Identity,
                                         bias=ebias[:], scale=SCALE)
                    pk = ps_sm.tile([P, 512], f32, tag="pp")
                    for kk in range(8):
                        nc.tensor.matmul(pk[:], wk_sb[:, kk, :],
                                         kt_sb[:, kk, sl],
                                         start=(kk == 0), stop=(kk == 7))
                    nc.scalar.activation(ktc[:, sl], pk[:], AF.Copy)

                # dots: d0/d1 per (head, s-chunk)
                for h in range(2):
                    hsl = slice(h * HD, (h + 1) * HD)
                    for i in range(8):
                        pd = ps_sm.tile([P, 512], f32, tag="pp")
                        nc.tensor.matmul(pd[:, :2],
                                         newqt[hsl, i * P:(i + 1) * P],
                                         enc2[hsl, :], start=True, stop=True)
                        nc.vector.tensor_copy(dots_sb[:, i, 2 * h:2 * h + 2],
                                              pd[:, :2])

                # V projection
                for m in range(8):
                    msl = slice(m * P, (m + 1) * P)
                    pv = ps_sm.tile([P, 512], f32, tag="pp")
                    for kk in range(8):
                        nc.tensor.matmul(pv[:, :P], vt_sb[:, kk, msl],
                                         wv_sb[:, kk, :],
                                         start=(kk == 0), stop=(kk == 7))
                    nc.vector.tensor_copy(v_sb[:, m, :], pv[:, :P])


            # ---------- phase 1+2: scores/softmax/transpose, split in halves
            def scores_iter(i, h):
                hsl = slice(h * HD, (h + 1) * HD)
                d0c = dots_sb[:, i, 2 * h:2 * h + 1]
                ddc = dots_sb[:, i, 2 * h + 1:2 * h + 2]
                dg0 = work.tile([P, P], bf16, tag="dg0")
                nc.vector.tensor_scalar(dg0[:], ident[:], d0c, None, ALU.mult)
                dgb = work.tile([P, P], bf16, tag="dgb")
                nc.scalar.activation(dgb[:], ident[:], AF.Copy, scale=ddc)

                sm = work.tile([P, S], fp16, tag="sm", bufs=4)
                ps_s = ps_big.tile([P, S], f32, tag="scores", bufs=2)
                for j in range(2):
                    sl = slice(j * 512, (j + 1) * 512)
                    nc.tensor.matmul(ps_s[:, sl],
                                     newqt[hsl, i * P:(i + 1) * P],
                                     ktc[hsl, sl], start=True, stop=False)
                    nc.tensor.matmul(ps_s[:, sl], dg0[:], utt_sb[:, i, sl],
                                     start=False, stop=False)
                    nc.tensor.matmul(ps_s[:, sl], dgb[:], w_sb[:, i, sl],
                                     start=False, stop=True)
                # mask + evict PSUM in one fused op -> fp16 SBUF
                nc.vector.scalar_tensor_tensor(sm[:], ps_s[:], 1.0,
                                               kp_sb[:, 8 * h + i, :],
                                               ALU.mult, ALU.mult)
                pn = work.tile([P, S], bf16, tag="pn", bufs=4)
                zc = work.tile([P, 1], f32, tag="zc", bufs=3)
                nc.scalar.activation(pn[:], sm[:], AF.Exp, accum_out=zc[:])
                zr = work.tile([P, 1], f32, tag="zr", bufs=3)
                nc.vector.reciprocal(zr[:], zc[:])
                pn2 = work.tile([P, S], bf16, tag="pn2", bufs=4)
                nc.vector.tensor_scalar(pn2[:], pn[:], zr[:], None, ALU.mult)
                nc.sync.dma_start_transpose(pts[h][:, :, i * P:(i + 1) * P],
                                            pn2[:])

            def pv_half(g):
                gs = slice(g * 512, (g + 1) * 512)
                ps_at0 = ps_o.tile([HD, 512], f32, tag="at0")
                ps_at1 = ps_o.tile([HD, 512], f32, tag="at1")
                ps_at = (ps_at0, ps_at1)
                for tcn in range(8):
                    for h in range(2):
                        nc.tensor.matmul(ps_at[h][:, :],
                                         v_sb[:, tcn, h * HD:(h + 1) * HD],
                                         pts[h][:, tcn, gs],
                                         start=(tcn == 0), stop=(tcn == 7))
                ath = work.tile([P, 512], bf16, tag="ath", bufs=2)
                nc.vector.tensor_copy(ath[:HD, :], ps_at0[:])
                nc.vector.tensor_copy(ath[HD:, :], ps_at1[:])
                nc.scalar.dma_start(at_d[g][:], ath[:])
                nc.gpsimd.collective_compute(
                    "AllGather",
                    mybir.AluOpType.bypass,
                    replica_groups=[list(range(N_CORES))],
                    ins=[at_d[g].opt()],
                    outs=[ag_d[g].opt()],
                )

            def oproj_half(g):
                atg = work.tile([P, 8, 512], bf16, tag="atg", bufs=2)
                for a in range(8):
                    nc.scalar.dma_start(atg[:, a, :],
                                        ag_d[g][a * P:(a + 1) * P, :])
                pf = ps_sm.tile([P, 512], f32, tag="pp")
                for kk in range(8):
                    nc.tensor.matmul(pf[:], wo_sb[:, kk, :], atg[:, kk, :],
                                     start=(kk == 0), stop=(kk == 7))
                of = work.tile([P, 512], f32, tag="of", bufs=2)
                nc.vector.tensor_copy(of[:], pf[:])
                nc.scalar.dma_start(out_e[:, g * 512:(g + 1) * 512], of[:])

            for i in range(8):
                for h in range(2):
                    scores_iter(i, h)
                if i == 3:
                    pv_half(0)
            pv_half(1)
            oproj_half(0)
            oproj_half(1)

    nc.compile()
    _CACHE["nc"] = nc
    return nc


def _prep_inputs(q, k, v, mask, utt_idx, spk_idx, Wq, Wk, Wv, Wo, k_enc):
    """Layout-only host prep: transpose/reshape/cast into per-core shards."""
    bf = ml_dtypes.bfloat16

    def chunked(x, dtype):
        # [1024, N] -> [128, 8, N] with row r = kk*128 + p -> [p, kk, :]
        return np.ascontiguousarray(
            x.reshape(8, P, -1).transpose(1, 0, 2).astype(dtype))

    qt = chunked(np.ascontiguousarray(q.T), bf)
    kt = chunked(np.ascontiguousarray(k.T), bf)
    vt = chunked(np.ascontiguousarray(v.T), bf)
    utt = chunked(utt_idx, bf)
    spk = chunked(spk_idx, np.uint8)
    keep = ~mask
    kr = k_enc.reshape(2, H, HD)

    maps = []
    for c in range(N_CORES):
        rows = slice(c * P, (c + 1) * P)
        m = dict(
            qt=qt, kt=kt, vt=vt, utt=utt, spk=spk,
            wq=chunked(np.ascontiguousarray(Wq[rows, :].T), bf),
            wk=chunked(np.ascontiguousarray(Wk[rows, :].T), bf),
            wv=chunked(np.ascontiguousarray(Wv[rows, :].T), bf),
            wo=chunked(np.ascontiguousarray(Wo[rows, :].T), bf),
            kp=np.ascontiguousarray(
                keep[2 * c:2 * c + 2].reshape(2, 8, P, S)
                .transpose(2, 0, 1, 3).reshape(P, 16, S).astype(np.uint8)),
            enc=np.ascontiguousarray(
                np.stack([kr[0, 2 * c:2 * c + 2].reshape(P),
                          kr[1, 2 * c:2 * c + 2].reshape(P)],
                         axis=1).astype(bf)),
            encq=np.ascontiguousarray(
                kr[0, 2 * c:2 * c + 2].reshape(P, 1).astype(np.float32)),
        )
        maps.append(m)
    return maps


def kernel(q, k, v, mask, utt_idx, spk_idx, Wq, Wk, Wv, Wo, k_enc):
    global LAST_EXEC_NS
    from concourse.bass_utils import run_bass_kernel_spmd

    nc = _build()
    in_maps = _prep_inputs(np.asarray(q, np.float32), np.asarray(k, np.float32),
                           np.asarray(v, np.float32), np.asarray(mask),
                           np.asarray(utt_idx, np.float32), np.asarray(spk_idx),
                           np.asarray(Wq, np.float32), np.asarray(Wk, np.float32),
                           np.asarray(Wv, np.float32), np.asarray(Wo, np.float32),
                           np.asarray(k_enc, np.float32))
    res = run_bass_kernel_spmd(nc, in_maps, list(range(N_CORES)),
                               trace=TRACE, tmpdir=TRACE_DIR)
    LAST_EXEC_NS = res.exec_time_ns
    outT = np.concatenate([res.results[c]["out"] for c in range(N_CORES)],
                          axis=0)
    return np.ascontiguousarray(outT.T).astype(np.float32)


# revision 22
# speedup vs baseline: 1.0120x; 1.0120x over previous
"""Trainium2 Bass kernel for nn_AttentionType1 (S=1024, E=1024, H=16, HD=64).

Tensor-parallel over heads, 2 heads per core on 8 NeuronCores.

Per core c (heads 2c, 2c+1):
  - Projections (bf16, weight slices stationary): newQT = (Wq_c @ q.T + q_emb)*scale,
    KT = Wk_c @ k.T (both [128, S], head-dim on partitions), V natural [t, d].
  - Scores [s, t] per (head, s-chunk) in PSUM: s1 via QK matmul (k=64 per head,
    both heads row-packed in the PE) plus the relative/speaker term folded into
    the TensorEngine as two diagonal-stationary matmuls:
    s2 = diag(d0) @ utt + diag(d1-d0) @ (spk*utt).
  - Mask + PSUM eviction fused: one scalar_tensor_tensor multiplies by keep
    (1-mask) while moving PSUM->SBUF fp16 (reference's 1e-30 equals 0.0 under
    exp in fp32).
  - Softmax: fused exp + row-sum on ScalarE (accum_out), no max-subtraction
    (logits bounded ~|8|), normalize P by 1/Z per-partition (tensor_scalar 4x).
  - P transposed via DMA-xbar transpose (bf16) into [t', tc, s] tiles.
  - PV: V tiles stationary, both heads packed into one PSUM via column tiling.
  - Output: AllGather the tiny attn_out.T (bf16) in two s-halves; AG of half 0
    overlaps the scores of s-chunks 4..7. Each core then computes a distinct
    128-row slice of out.T = Wo @ attn_out.T locally -- no all-reduce.
Host does layout-only prep (transpose/reshape/cast) and concatenation.
"""

import sys

if "/opt/trn_rl_repo" not in sys.path:
    sys.path.insert(0, "/opt/trn_rl_repo")

import numpy as np
import ml_dtypes

S = 1024
E = 1024
H = 16
HD = 64
N_CORES = 8
P = 128
SCALE = float(HD) ** -0.5  # 0.125

_CACHE = {}
LAST_EXEC_NS = None
TRACE = False
TRACE_DIR = None


def _build():
    if "nc" in _CACHE:
        return _CACHE["nc"]

    import concourse.mybir as mybir
    import concourse.tile as tile
    from concourse import bacc
    from concourse.masks import make_identity

    f32 = mybir.dt.float32
    bf16 = mybir.dt.bfloat16
    fp16 = mybir.dt.float16
    u8 = mybir.dt.uint8
    AF = mybir.ActivationFunctionType
    ALU = mybir.AluOpType

    nc = bacc.Bacc("TRN2", target_bir_lowering=False, debug=False,
                   num_devices=N_CORES)

    # --- external IO (per-core shards, host-prepped layouts) ---
    qt_e = nc.dram_tensor("qt", [P, 8, S], bf16, kind="ExternalInput").ap()
    kt_e = nc.dram_tensor("kt", [P, 8, S], bf16, kind="ExternalInput").ap()
    vt_e = nc.dram_tensor("vt", [P, 8, S], bf16, kind="ExternalInput").ap()
    wq_e = nc.dram_tensor("wq", [P, 8, P], bf16, kind="ExternalInput").ap()
    wk_e = nc.dram_tensor("wk", [P, 8, P], bf16, kind="ExternalInput").ap()
    wv_e = nc.dram_tensor("wv", [P, 8, P], bf16, kind="ExternalInput").ap()
    wo_e = nc.dram_tensor("wo", [P, 8, P], bf16, kind="ExternalInput").ap()
    utt_e = nc.dram_tensor("utt", [P, 8, S], bf16, kind="ExternalInput").ap()
    spk_e = nc.dram_tensor("spk", [P, 8, S], u8, kind="ExternalInput").ap()
    kp_e = nc.dram_tensor("kp", [P, 16, S], u8, kind="ExternalInput").ap()
    enc_e = nc.dram_tensor("enc", [P, 2], bf16, kind="ExternalInput").ap()
    encq_e = nc.dram_tensor("encq", [P, 1], f32, kind="ExternalInput").ap()
    out_e = nc.dram_tensor("out", [P, S], f32, kind="ExternalOutput").ap()

    class _NoAddSet(set):
        def add(self, x):  # noqa: ARG002
            pass

    with tile.TileContext(nc) as tc:
        # Collectives here only touch DRAM buffers that no DMA-transpose ever
        # reads or writes; skip the global transpose<->collective
        # serialization, which otherwise stalls the softmax pipeline behind
        # every AllGather.
        tc.serialize_transpose_collective_names = _NoAddSet()
        with tc.tile_pool(name="const", bufs=1) as const, \
             tc.tile_pool(name="pers", bufs=1) as pers, \
             tc.tile_pool(name="work", bufs=2) as work, \
             tc.tile_pool(name="ps_big", bufs=2, space="PSUM") as ps_big, \
             tc.tile_pool(name="ps_sm", bufs=2, space="PSUM") as ps_sm, \
             tc.tile_pool(name="ps_o", bufs=1, space="PSUM") as ps_o, \
             tc.tile_pool(name="dram", bufs=1, space="DRAM") as dram:

            ident = const.tile([P, P], bf16)
            make_identity(nc, ident[:])
            enc_sb = const.tile([P, 2], bf16)
            nc.sync.dma_start(enc_sb[:], enc_e[:])
            encq_sb = const.tile([P, 1], f32)
            nc.sync.dma_start(encq_sb[:], encq_e[:])
            ebias = const.tile([P, 1], f32)
            nc.vector.tensor_scalar_mul(ebias[:], encq_sb[:], SCALE)
            enc2 = const.tile([P, 2], bf16)
            nc.vector.tensor_copy(enc2[:, 0:1], enc_sb[:, 0:1])
            nc.vector.tensor_sub(enc2[:, 1:2], enc_sb[:, 1:2], enc_sb[:, 0:1])

            newqt = pers.tile([P, S], bf16)
            ktc = pers.tile([P, S], bf16)
            v_sb = pers.tile([P, 8, P], bf16)      # [t', tc, d(2 heads)]
            utt_sb = pers.tile([P, 8, S], bf16)    # [p, i, t], s = i*128+p
            w_sb = pers.tile([P, 8, S], bf16)      # spk*utt
            kp_sb = pers.tile([P, 16, S], u8)      # keep = 1-mask, [p, 8h+i, t]
            dots_sb = pers.tile([P, 8, 4], f32)    # [p, i, 2h+v]
            wo_sb = pers.tile([P, 8, P], bf16)
            pt0 = pers.tile([P, 8, S], bf16)       # P.T head0: [t', tc, s]
            pt1 = pers.tile([P, 8, S], bf16)
            pts = (pt0, pt1)

            # tiny warmup AllGather (content irrelevant): absorbs the
            # collectives first-call cost under the input-DMA phase
            wu_in = dram.tile([P, 2], bf16, name="wu_in")
            wu_out = dram.tile([N_CORES * P, 2], bf16, addr_space="Shared",
                               name="wu_out")
            nc.gpsimd.collective_compute(
                "AllGather", mybir.AluOpType.bypass,
                replica_groups=[list(range(N_CORES))],
                ins=[wu_in.opt()], outs=[wu_out.opt()])
            # DRAM bounce buffers for the two AllGather halves
            at_d = [dram.tile([P, 512], bf16, name=f"at_d{g}") for g in range(2)]
            ag_d = [dram.tile([N_CORES * P, 512], bf16, addr_space="Shared",
                              name=f"ag_d{g}") for g in range(2)]

            # ---------- input DMAs (ordered by first use) ----------
            with tc.tile_pool(name="setup", bufs=1) as setup:
                # sync ring: ONLY the q/k projection prefix -- it gates the
                # whole scores pipeline
                wq_sb = setup.tile([P, 8, P], bf16)
                nc.sync.dma_start(wq_sb[:], wq_e[:])
                wk_sb = setup.tile([P, 8, P], bf16)
                nc.sync.dma_start(wk_sb[:], wk_e[:])
                qt_sb = setup.tile([P, 8, S], bf16)
                nc.sync.dma_start(qt_sb[:], qt_e[:])
                kt_sb = setup.tile([P, 8, S], bf16)
                nc.sync.dma_start(kt_sb[:], kt_e[:])
                spk_sb = setup.tile([P, 8, S], u8)
                vt_sb = setup.tile([P, 8, S], bf16)
                wv_sb = setup.tile([P, 8, P], bf16)
                # scalar ring: mask/speaker inputs per s-chunk by first use;
                # V after the first-half masks
                for i in range(8):
                    nc.scalar.dma_start(utt_sb[:, i:i + 1, :],
                                        utt_e[:, i:i + 1, :])
                    nc.scalar.dma_start(spk_sb[:, i:i + 1, :],
                                        spk_e[:, i:i + 1, :])
                    nc.scalar.dma_start(kp_sb[:, i:i + 1, :],
                                        kp_e[:, i:i + 1, :])
                    nc.scalar.dma_start(kp_sb[:, 8 + i:9 + i, :],
                                        kp_e[:, 8 + i:9 + i, :])
                    nc.gpsimd.tensor_mul(w_sb[:, i, :], spk_sb[:, i, :],
                                         utt_sb[:, i, :])
                    if i == 3:
                        nc.scalar.dma_start(vt_sb[:], vt_e[:])
                        nc.scalar.dma_start(wv_sb[:], wv_e[:])
                nc.scalar.dma_start(wo_sb[:], wo_e[:])

                # ---------- phase 0: projections ----------
                for n in range(2):
                    sl = slice(n * 512, (n + 1) * 512)
                    pq = ps_sm.tile([P, 512], f32, tag="pp")
                    for kk in range(8):
                        nc.tensor.matmul(pq[:], wq_sb[:, kk, :],
                                         qt_sb[:, kk, sl],
                                         start=(kk == 0), stop=(kk == 7))
                    nc.scalar.activation(newqt[:, sl], pq[:], AF.Identity,
                                         bias=ebias[:], scale=SCALE)
                    pk = ps_sm.tile([P, 512], f32, tag="pp")
                    for kk in range(8):
                        nc.tensor.matmul(pk[:], wk_sb[:, kk, :],
                                         kt_sb[:, kk, sl],
                                         start=(kk == 0), stop=(kk == 7))
                    nc.scalar.activation(ktc[:, sl], pk[:], AF.Copy)

                # dots: d0/d1 per (head, s-chunk)
                for h in range(2):
                    hsl = slice(h * HD, (h + 1) * HD)
                    for i in range(8):
                        pd = ps_sm.tile([P, 512], f32, tag="pp")
                        nc.tensor.matmul(pd[:, :2],
                                         newqt[hsl, i * P:(i + 1) * P],
                                         enc2[hsl, :], start=True, stop=True)
                        nc.vector.tensor_copy(dots_sb[:, i, 2 * h:2 * h + 2],
                                              pd[:, :2])

                # V projection
                for m in range(8):
                    msl = slice(m * P, (m + 1) * P)
                    pv = ps_sm.tile([P, 512], f32, tag="pp")
                    for kk in range(8):
                        nc.tensor.matmul(pv[:, :P], vt_sb[:, kk, msl],
                                         wv_sb[:, kk, :],
                                         start=(kk == 0), stop=(kk == 7))
                    nc.vector.tensor_copy(v_sb[:, m, :], pv[:, :P])


            # ---------- phase 1+2: scores/softmax/transpose, split in halves
            def scores_iter(i, h):
                hsl = slice(h * HD, (h + 1) * HD)
                d0c = dots_sb[:, i, 2 * h:2 * h + 1]
                ddc = dots_sb[:, i, 2 * h + 1:2 * h + 2]
                dg0 = work.tile([P, P], bf16, tag="dg0")
                nc.vector.tensor_scalar(dg0[:], ident[:], d0c, None, ALU.mult)
                dgb = work.tile([P, P], bf16, tag="dgb")
                nc.scalar.activation(dgb[:], ident[:], AF.Copy, scale=ddc)

                sm = work.tile([P, S], fp16, tag="sm", bufs=4)
                ps_s = ps_big.tile([P, S], f32, tag="scores", bufs=2)
                for j in range(2):
                    sl = slice(j * 512, (j + 1) * 512)
                    nc.tensor.matmul(ps_s[:, sl],
                                     newqt[hsl, i * P:(i + 1) * P],
                                     ktc[hsl, sl], start=True, stop=False)
                    nc.tensor.matmul(ps_s[:, sl], dg0[:], utt_sb[:, i, sl],
                                     start=False, stop=False)
                    nc.tensor.matmul(ps_s[:, sl], dgb[:], w_sb[:, i, sl],
                                     start=False, stop=True)
                # mask + evict PSUM in one fused op -> fp16 SBUF
                nc.vector.scalar_tensor_tensor(sm[:], ps_s[:], 1.0,
                                               kp_sb[:, 8 * h + i, :],
                                               ALU.mult, ALU.mult)
                pn = work.tile([P, S], bf16, tag="pn", bufs=4)
                zc = work.tile([P, 1], f32, tag="zc", bufs=3)
                nc.scalar.activation(pn[:], sm[:], AF.Exp, accum_out=zc[:])
                zr = work.tile([P, 1], f32, tag="zr", bufs=3)
                nc.vector.reciprocal(zr[:], zc[:])
                pn2 = work.tile([P, S], bf16, tag="pn2", bufs=4)
                nc.vector.tensor_scalar(pn2[:], pn[:], zr[:], None, ALU.mult)
                nc.sync.dma_start_transpose(pts[h][:, :, i * P:(i + 1) * P],
                                            pn2[:])

            def pv_half(g):
                gs = slice(g * 512, (g + 1) * 512)
                ps_at0 = ps_o.tile([HD, 512], f32, tag="at0")
                ps_at1 = ps_o.tile([HD, 512], f32, tag="at1")
                ps_at = (ps_at0, ps_at1)
                for tcn in range(8):
                    for h in range(2):
                        nc.tensor.matmul(ps_at[h][:, :],
                                         v_sb[:, tcn, h * HD:(h + 1) * HD],
                                         pts[h][:, tcn, gs],
                                         start=(tcn == 0), stop=(tcn == 7))
                ath = work.tile([P, 512], bf16, tag="ath", bufs=2)
                nc.vector.tensor_copy(ath[:HD, :], ps_at0[:])
                nc.vector.tensor_copy(ath[HD:, :], ps_at1[:])
                nc.scalar.dma_start(at_d[g][:], ath[:])
                nc.gpsimd.collective_compute(
                    "AllGather",
                    mybir.AluOpType.bypass,
                    replica_groups=[list(range(N_CORES))],
                    ins=[at_d[g].opt()],
                    outs=[ag_d[g].opt()],
                )

            def oproj_half(g):
                atg = work.tile([P, 8, 512], bf16, tag="atg", bufs=2)
                for a in range(8):
                    nc.scalar.dma_start(atg[:, a, :],
                                        ag_d[g][a * P:(a + 1) * P, :])
                pf = ps_sm.tile([P, 512], f32, tag="pp")
                for kk in range(8):
                    nc.tensor.matmul(pf[:], wo_sb[:, kk, :], atg[:, kk, :],
                                     start=(kk == 0), stop=(kk == 7))
                of = work.tile([P, 512], f32, tag="of", bufs=2)
                nc.vector.tensor_copy(of[:], pf[:])
                nc.scalar.dma_start(out_e[:, g * 512:(g + 1) * 512], of[:])

            for i in range(8):
                for h in range(2):
                    scores_iter(i, h)
                if i == 3:
                    pv_half(0)
            pv_half(1)
            oproj_half(0)
            oproj_half(1)

    nc.compile()
    _CACHE["nc"] = nc
    return nc


def _prep_inputs(q, k, v, mask, utt_idx, spk_idx, Wq, Wk, Wv, Wo, k_enc):
    """Layout-only host prep: transpose/reshape/cast into per-core shards."""
    bf = ml_dtypes.bfloat16

    def chunked(x, dtype):
        # [1024, N] -> [128, 8, N] with row r = kk*128 + p -> [p, kk, :]
        return np.ascontiguousarray(
            x.reshape(8, P, -1).transpose(1, 0, 2).astype(dtype))

    qt = chunked(np.ascontiguousarray(q.T), bf)
    kt = chunked(np.ascontiguousarray(k.T), bf)
    vt = chunked(np.ascontiguousarray(v.T), bf)
    utt = chunked(utt_idx, bf)
    spk = chunked(spk_idx, np.uint8)
    keep = ~mask
    kr = k_enc.reshape(2, H, HD)

    maps = []
    for c in range(N_CORES):
        rows = slice(c * P, (c + 1) * P)
        m = dict(
            qt=qt, kt=kt, vt=vt, utt=utt, spk=spk,
            wq=chunked(np.ascontiguousarray(Wq[rows, :].T), bf),
            wk=chunked(np.ascontiguousarray(Wk[rows, :].T), bf),
            wv=chunked(np.ascontiguousarray(Wv[rows, :].T), bf),
            wo=chunked(np.ascontiguousarray(Wo[rows, :].T), bf),
            kp=np.ascontiguousarray(
                keep[2 * c:2 * c + 2].reshape(2, 8, P, S)
                .transpose(2, 0, 1, 3).reshape(P, 16, S).astype(np.uint8)),
            enc=np.ascontiguousarray(
                np.stack([kr[0, 2 * c:2 * c + 2].reshape(P),
                          kr[1, 2 * c:2 * c + 2].reshape(P)],
                         axis=1).astype(bf)),
            encq=np.ascontiguousarray(
                kr[0, 2 * c:2 * c + 2].reshape(P, 1).astype(np.float32)),
        )
        maps.append(m)
    return maps


def kernel(q, k, v, mask, utt_idx, spk_idx, Wq, Wk, Wv, Wo, k_enc):
    global LAST_EXEC_NS
    from concourse.bass_utils import run_bass_kernel_spmd

    nc = _build()
    in_maps = _prep_inputs(np.asarray(q, np.float32), np.asarray(k, np.float32),
                           np.asarray(v, np.float32), np.asarray(mask),
                           np.asarray(utt_idx, np.float32), np.asarray(spk_idx),
                           np.asarray(Wq, np.float32), np.asarray(Wk, np.float32),
                           np.asarray(Wv, np.float32), np.asarray(Wo, np.float32),
                           np.asarray(k_enc, np.float32))
    res = run_bass_kernel_spmd(nc, in_maps, list(range(N_CORES)),
                               trace=TRACE, tmpdir=TRACE_DIR)
    LAST_EXEC_NS = res.exec_time_ns
    outT = np.concatenate([res.results[c]["out"] for c in range(N_CORES)],
                          axis=0)
    return np.ascontiguousarray(outT.T).astype(np.float32)
